# revision 5
# baseline (speedup 1.0000x reference)
"""Multi-head attention (B=2, H=16, S=2048, Dh=64) on 8 trn2 NeuronCores.

Sharding: core c handles batch c//4, heads (c%4)*4 .. +4. Each core computes
attention for its 4 (b,h) pairs independently (no collectives).

Device algorithm per (head, q-chunk of 512), k-tiles grouped in triples so
the ScalarE exp runs as 96 wide ACTIVATEs (FD 1536/512) instead of 128:
  S^T[k,q] = K @ Q^T   (bf16 matmuls; each k-tile's 64-deep contraction sits
                        in PE rows (j%2)*64 so adjacent matmuls row-tile)
  att      = exp(0.125 * S^T)       (ScalarE, PSUM->SBUF bf16 — the
                                     critical-path engine, ~1 elem/lane/cy)
  att     *= mask^T                 (VectorE bf16, in place, so the ACTIVATE
                                     needs only one deduped PE-sem wait)
  O^T[d,q] += [V | 1]^T @ att       (PSUM accumulation over all 16 k-tiles;
                                     the ones column makes row 64 the sums)
Host: normalize O^T rows by the sums row and transpose back to [S, Dh].
"""

import numpy as np
import ml_dtypes

TRACE = False
LAST = {}

B, H, S, Dh = 2, 16, 2048, 64
HPC = 4            # heads per core
NCORES = 8
SCALE = 0.125      # Dh ** -0.5
KT = 16            # k tiles of 128
KTP = 8            # k tile pairs (kT2 column blocks)
QC = 4             # q chunks of 512
BF16 = ml_dtypes.bfloat16

_NC = None


def _dedupe_and_split_waits(nc, max_waits=1):
    """Collapse same-semaphore sem-ge-imm waits to the max threshold (they
    are monotonic counters, so the max implies the rest), then move any
    remaining excess onto NoOps inserted just before on the same engine —
    this container's walrus rejects >max_waits semaphore waits on one
    instruction (CoreV3 setupSyncWait "Too many sync wait commands")."""
    import concourse.mybir as mybir

    ctr = 0
    for f in nc.m.functions:
        for b in f.blocks:
            insts = b.instructions
            new = []
            for inst in insts:
                si = inst.sync_info
                waits = list(si.on_wait) if si else []
                if len(waits) > 1:
                    best, other = {}, []
                    for w in waits:
                        if w.wait_mode == "sem-ge-imm" and w.wait_reg is None:
                            k = (w.sync_type, w.id)
                            if k not in best or w.wait_value > best[k].wait_value:
                                best[k] = w
                        else:
                            other.append(w)
                    waits = list(best.values()) + other
                if len(waits) > max_waits:
                    for w in waits[:-max_waits]:
                        ctr += 1
                        new.append(
                            mybir.InstNoOp(
                                name=f"waitsplit-{ctr}",
                                engine=inst.engine,
                                ins=[],
                                outs=[],
                                sync_info=mybir.SyncInfo(on_wait=[w], on_update=[]),
                            )
                        )
                    waits = waits[-max_waits:]
                if si:
                    inst.sync_info = mybir.SyncInfo(
                        on_wait=waits, on_update=list(si.on_update)
                    )
                new.append(inst)
            insts[:] = new


def _patch_minimal_teardown():
    """Tile's exit emits drain + 2 all-engine barriers + semaphore clears
    (~10us on the critical tail). The barriers/clears only matter for
    re-executing an already-loaded NEFF; each kernel() call loads fresh, so
    keep just the final drain (it carries the waits that guarantee the
    output DMAs completed)."""
    import concourse.tile as tile
    from concourse.vector_clock import ScopedClock

    if getattr(tile.TileContext._drain_and_barrier, "_minimal", False):
        return

    def _drain_and_barrier(self, tick_clock, wait_clock):
        drain_inst = self.nc.sync.drain()
        wait_clock.add_sem_waits(
            drain_inst.ins, ScopedClock({None: tick_clock.global_clock})
        )
        popped = self.nc._tile_sem_poison_stack.pop()
        assert popped is self._sem_poison

    _drain_and_barrier._minimal = True
    tile.TileContext._drain_and_barrier = _drain_and_barrier


def _build_nc():
    import concourse.bass as bass
    import concourse.mybir as mybir
    import concourse.tile as tile

    _patch_minimal_teardown()
    bf = mybir.dt.bfloat16
    f32 = mybir.dt.float32
    Exp = mybir.ActivationFunctionType.Exp

    nc = bass.Bass()
    qT2_e = nc.declare_dram_parameter("qT2", [HPC, 128, S], bf, isOutput=False)
    kT2_e = nc.declare_dram_parameter("kT2", [HPC, 128, KTP * 128], bf, isOutput=False)
    vh_e = nc.declare_dram_parameter("vh", [HPC, 128, KT * 65], bf, isOutput=False)
    mF_e = nc.declare_dram_parameter("maskF", [QC, 128, KT * 512], bf, isOutput=False)
    out_e = nc.declare_dram_parameter("out", [HPC, 65, S], bf, isOutput=True)

    with tile.TileContext(nc) as tc:
        with (
            tc.tile_pool(name="maskA", bufs=20) as maskA,
            tc.tile_pool(name="maskC", bufs=4) as maskC,
            tc.tile_pool(name="qp", bufs=4) as qp,
            tc.tile_pool(name="kp", bufs=4) as kp,
            tc.tile_pool(name="vp", bufs=4) as vp,
            tc.tile_pool(name="attA", bufs=4) as attAp,
            tc.tile_pool(name="attC", bufs=2) as attCp,
            tc.tile_pool(name="obp", bufs=2) as obp,
            tc.tile_pool(name="wp", bufs=1) as wp,
            tc.tile_pool(name="pA", bufs=2, space=bass.MemorySpace.PSUM) as pA,
            tc.tile_pool(name="pC", bufs=1, space=bass.MemorySpace.PSUM) as pC,
            tc.tile_pool(name="pO", bufs=1, space=bass.MemorySpace.PSUM) as pO,
        ):
            # touch the Exp table before any data arrives so the one-time
            # ACT_TABLE_LOAD overlaps the DMA ramp instead of the first tile
            warm_in = wp.tile([128, 8], bf, name="warm_in")
            nc.gpsimd.memset(warm_in[:], 0.0)
            warm_out = wp.tile([128, 8], bf, name="warm_out")
            nc.scalar.activation(warm_out[:], warm_in[:], Exp, scale=1.0)

            # DMA issue order = first-use order so the pipeline starts as
            # early as possible during the ramp: k0, first q half, first
            # mask tile, rest of head 0, then masks ahead of their heads.
            kts = [kp.tile([128, KTP * 128], bf, tag=f"k{h}", name=f"kt{h}", bufs=1)
                   for h in range(HPC)]
            qts = [qp.tile([128, S], bf, tag=f"q{h}", name=f"qt{h}", bufs=1)
                   for h in range(HPC)]
            vts = [vp.tile([128, KT * 65], bf, tag=f"v{h}", name=f"vt{h}", bufs=1)
                   for h in range(HPC)]
            mtiles = [[None] * 6 for _ in range(QC)]

            def load_mask(qq, t):
                if t < 5:
                    mt = maskA.tile([128, 1536], bf, tag=f"mA{qq}_{t}",
                                    name=f"mA{qq}_{t}", bufs=1)
                    nc.sync.dma_start(mt[:], mF_e[qq, :, t * 1536:(t + 1) * 1536])
                else:
                    mt = maskC.tile([128, 512], bf, tag=f"mC{qq}",
                                    name=f"mC{qq}", bufs=1)
                    nc.sync.dma_start(mt[:], mF_e[qq, :, 7680:8192])
                mtiles[qq][t] = mt

            nc.sync.dma_start(kts[0][:], kT2_e[0])
            nc.sync.dma_start(qts[0][:, 0:1024], qT2_e[0, :, 0:1024])
            load_mask(0, 0)
            nc.sync.dma_start(qts[0][:, 1024:2048], qT2_e[0, :, 1024:2048])
            nc.sync.dma_start(vts[0][:], vh_e[0])
            for t in range(1, 6):
                load_mask(0, t)
            for t in range(6):
                load_mask(1, t)
            nc.sync.dma_start(kts[1][:], kT2_e[1])
            nc.sync.dma_start(qts[1][:], qT2_e[1])
            nc.sync.dma_start(vts[1][:], vh_e[1])
            for t in range(6):
                load_mask(2, t)
            nc.sync.dma_start(kts[2][:], kT2_e[2])
            nc.sync.dma_start(qts[2][:], qT2_e[2])
            nc.sync.dma_start(vts[2][:], vh_e[2])
            for t in range(6):
                load_mask(3, t)
            nc.sync.dma_start(kts[3][:], kT2_e[3])
            nc.sync.dma_start(qts[3][:], qT2_e[3])
            nc.sync.dma_start(vts[3][:], vh_e[3])

            def s_matmul(s, kt_, qt, qq, j, col):
                rh = (j % 2) * 64
                nc.tensor.matmul(
                    s[:, col * 512:(col + 1) * 512],
                    kt_[rh:rh + 64, (j // 2) * 128:(j // 2 + 1) * 128],
                    qt[rh:rh + 64, qq * 512:(qq + 1) * 512],
                    start=True,
                    stop=True,
                )

            for h in range(HPC):
                qt, kt_, vt = qts[h], kts[h], vts[h]
                for qq in range(QC):
                    o_ps = pO.tile([65, 512], f32, name=f"o_{h}_{qq}", tag="o")
                    # k-tile 15's S matmul is emitted first: its PSUM bank
                    # has been free since the previous group's exp, so it
                    # runs early instead of serializing behind the whole
                    # tail chain of the group (accumulation order over
                    # k-tiles is arbitrary).
                    sc = pC.tile([128, 512], f32, name=f"sc{h}_{qq}", tag="sC")
                    s_matmul(sc, kt_, qt, qq, 15, 0)
                    for t in range(5):
                        s = pA.tile([128, 1536], f32, name=f"s{h}_{qq}_{t}",
                                    tag="sA")
                        att = attAp.tile([128, 1536], bf, tag="attA",
                                         name="attA")
                        J = (3 * t, 3 * t + 1, 3 * t + 2)
                        for idx, j in enumerate(J):
                            s_matmul(s, kt_, qt, qq, j, idx)
                        nc.scalar.activation(att[:], s[:], Exp, scale=SCALE)
                        nc.vector.tensor_mul(att[:], att[:], mtiles[qq][t][:])
                        for idx, j in enumerate(J):
                            nc.tensor.matmul(
                                o_ps[:],
                                vt[:, j * 65:(j + 1) * 65],
                                att[:, idx * 512:(idx + 1) * 512],
                                start=(j == 0),
                                stop=False,
                            )
                    attc = attCp.tile([128, 512], bf, tag="attC", name="attC")
                    nc.scalar.activation(attc[:], sc[:], Exp, scale=SCALE)
                    nc.vector.tensor_mul(attc[:], attc[:], mtiles[qq][5][:])
                    nc.tensor.matmul(
                        o_ps[:],
                        vt[:, 15 * 65:16 * 65],
                        attc[:],
                        start=False,
                        stop=True,
                    )
                    o_sb = obp.tile([65, 512], bf, name=f"osb_{h}_{qq}", tag="osb")
                    nc.vector.tensor_copy(o_sb[:], o_ps[:])
                    nc.sync.dma_start(
                        out_e[h, :, qq * 512:(qq + 1) * 512], o_sb[:]
                    )
    import os

    _dedupe_and_split_waits(nc, max_waits=int(os.environ.get("KMAXW", "1")))
    return nc


def _core_inputs(q, k, v, mask, core):
    b = core // HPC
    h0 = (core % HPC) * HPC
    qh = q[b, h0:h0 + HPC].transpose(0, 2, 1)            # [4, 64, S]
    qT2 = np.concatenate([qh, qh], axis=1)               # [4, 128, S]
    kh = k[b, h0:h0 + HPC].transpose(0, 2, 1)            # [4, 64, S]
    kT2 = (
        kh.reshape(HPC, 64, KTP, 2, 128)
        .transpose(0, 3, 1, 2, 4)
        .reshape(HPC, 128, KTP * 128)
    )
    vv = v[b, h0:h0 + HPC]                               # [4, S, 64]
    vh = np.concatenate(
        [vv, np.ones((HPC, S, 1), dtype=np.float32)], axis=2
    )                                                    # [4, S, 65]
    vh = vh.reshape(HPC, KT, 128, 65).transpose(0, 2, 1, 3).reshape(HPC, 128, KT * 65)
    mT = np.ascontiguousarray(mask[b, 0].T)              # [k, q]
    mF = (
        mT.reshape(KT, 128, QC, 512)
        .transpose(2, 1, 0, 3)
        .reshape(QC, 128, KT * 512)
    )
    return {
        "qT2": np.ascontiguousarray(qT2).astype(BF16),
        "kT2": np.ascontiguousarray(kT2).astype(BF16),
        "vh": np.ascontiguousarray(vh).astype(BF16),
        "maskF": np.ascontiguousarray(mF).astype(BF16),
    }


def kernel(q, k, v, mask):
    global _NC
    from concourse.bass_utils import run_bass_kernel_spmd

    q = np.asarray(q, dtype=np.float32)
    k = np.asarray(k, dtype=np.float32)
    v = np.asarray(v, dtype=np.float32)
    mask = np.asarray(mask)

    in_maps = [_core_inputs(q, k, v, mask, c) for c in range(NCORES)]
    if _NC is None:
        _NC = _build_nc()

    res = run_bass_kernel_spmd(
        _NC, in_maps, core_ids=list(range(NCORES)), trace=TRACE
    )
    LAST["exec_time_ns"] = res.exec_time_ns
    LAST["results"] = res

    out = np.empty((B, H, S, Dh), dtype=np.float32)
    for c in range(NCORES):
        b = c // HPC
        h0 = (c % HPC) * HPC
        o = np.asarray(res.results[c]["out"], dtype=np.float32)  # [4, 65, S]
        sums = o[:, Dh:Dh + 1, :]                        # [4, 1, S]
        on = o[:, :Dh, :] / sums                         # [4, 64, S]
        out[b, h0:h0 + HPC] = on.transpose(0, 2, 1)
    return out


# revision 12
# speedup vs baseline: 1.1823x; 1.1823x over previous
"""Multi-head attention (B=2, H=16, S=2048, Dh=64) on 8 trn2 NeuronCores.

Sharding: core c handles batch c//4, heads (c%4)*4 .. +4. Each core computes
attention for its 4 (b,h) pairs independently (no collectives).

Device algorithm per (head, q-chunk of 512), k-tiles grouped in triples so
the ScalarE exp runs as 96 wide ACTIVATEs (FD 1536/512) instead of 128:
  S^T[k,q] = K @ Q^T   (bf16 matmuls; each k-tile's 64-deep contraction sits
                        in PE rows (j%2)*64 so adjacent matmuls row-tile)
  att      = exp(0.125 * S^T)       (ScalarE, PSUM->SBUF bf16 — the
                                     critical-path engine, ~1 elem/lane/cy)
  att     *= mask^T                 (VectorE bf16, in place, so the ACTIVATE
                                     needs only one deduped PE-sem wait)
  O^T[d,q] += [V | 1]^T @ att       (PSUM accumulation over all 16 k-tiles;
                                     the ones column makes row 64 the sums)
Host: normalize O^T rows by the sums row and transpose back to [S, Dh].
"""

import math
import os

import numpy as np
import ml_dtypes

TRACE = False
LAST = {}

# A-tiles whose exp runs on VectorE via the Schraudolph bit-trick
# (i16 = round(s*128*SCALE/ln2 + (127*128 - 7.4)), reinterpreted as bf16
# ~ exp(SCALE*s) with ~1.8% rms error) instead of on the saturated ScalarE.
# Value = offloaded A-tiles per 5-tile group (0..5).
OFFLOAD = int(os.environ.get("KOFFLOAD", "1"))
SCHR_A = 128.0 / math.log(2.0)
SCHR_B = 127.0 * 128.0 - 7.4

B, H, S, Dh = 2, 16, 2048, 64
HPC = 4            # heads per core
NCORES = 8
SCALE = 0.125      # Dh ** -0.5
KT = 16            # k tiles of 128
KTP = 8            # k tile pairs (kT2 column blocks)
QC = 4             # q chunks of 512
BF16 = ml_dtypes.bfloat16

_NC = None


def _dedupe_and_split_waits(nc, max_waits=1):
    """Collapse same-semaphore sem-ge-imm waits to the max threshold (they
    are monotonic counters, so the max implies the rest), then move any
    remaining excess onto NoOps inserted just before on the same engine —
    this container's walrus rejects >max_waits semaphore waits on one
    instruction (CoreV3 setupSyncWait "Too many sync wait commands")."""
    import concourse.mybir as mybir

    ctr = 0
    for f in nc.m.functions:
        for b in f.blocks:
            insts = b.instructions
            new = []
            for inst in insts:
                si = inst.sync_info
                waits = list(si.on_wait) if si else []
                if len(waits) > 1:
                    best, other = {}, []
                    for w in waits:
                        if w.wait_mode == "sem-ge-imm" and w.wait_reg is None:
                            k = (w.sync_type, w.id)
                            if k not in best or w.wait_value > best[k].wait_value:
                                best[k] = w
                        else:
                            other.append(w)
                    waits = list(best.values()) + other
                if len(waits) > max_waits:
                    for w in waits[:-max_waits]:
                        ctr += 1
                        new.append(
                            mybir.InstNoOp(
                                name=f"waitsplit-{ctr}",
                                engine=inst.engine,
                                ins=[],
                                outs=[],
                                sync_info=mybir.SyncInfo(on_wait=[w], on_update=[]),
                            )
                        )
                    waits = waits[-max_waits:]
                if si:
                    inst.sync_info = mybir.SyncInfo(
                        on_wait=waits, on_update=list(si.on_update)
                    )
                new.append(inst)
            insts[:] = new


def _bundle_scalar_dve_waits(nc, runlen=4):
    """The ScalarE stream is [NoOp(wait DVE>=mul(t-6)), ACTIVATE(wait PE)]
    per tile — the NoOp carries the att-buffer WAW wait split off by the
    1-wait-per-instruction limit. Since the DVE semaphore is monotonic and
    the att pool is 6 deep, one NoOp waiting for the max of the next
    `runlen` thresholds (producer: the mul 6-runlen tiles back, strictly
    earlier on every chain, so no cycle) covers all of them. Collapses 96
    sem-wait evaluations on the critical-path engine down to ~24."""
    import concourse.mybir as mybir

    # The DVE engine semaphore: the one TensorTensor instructions bump.
    from collections import Counter

    dve_sems = Counter()
    for f in nc.m.functions:
        for b in f.blocks:
            for inst in b.instructions:
                if isinstance(inst, mybir.InstTensorTensor) and inst.sync_info:
                    for u in inst.sync_info.on_update:
                        dve_sems[u.id] += 1
    if not dve_sems:
        return
    dve_id = dve_sems.most_common(1)[0][0]

    def is_dve_nop(inst):
        return (
            isinstance(inst, mybir.InstNoOp)
            and inst.sync_info
            and len(inst.sync_info.on_wait) == 1
            and not inst.sync_info.on_update
            and inst.sync_info.on_wait[0].id == dve_id
            and inst.sync_info.on_wait[0].wait_mode == "sem-ge-imm"
        )

    for f in nc.m.functions:
        for b in f.blocks:
            insts = b.instructions
            seq = [
                i for i, inst in enumerate(insts)
                if inst.engine == mybir.EngineType.Activation
            ]
            # maximal chains of consecutive [NoOp(dve-wait), ACTIVATE] pairs
            chains, cur, k = [], [], 0
            while k < len(seq):
                inst = insts[seq[k]]
                if (
                    is_dve_nop(inst)
                    and k + 1 < len(seq)
                    and isinstance(insts[seq[k + 1]], mybir.InstActivation)
                ):
                    cur.append(seq[k])
                    k += 2
                else:
                    if cur:
                        chains.append(cur)
                        cur = []
                    k += 1
            if cur:
                chains.append(cur)
            drop = set()
            for chain in chains:
                for c0 in range(0, len(chain), runlen):
                    chunk = chain[c0:c0 + runlen]
                    if len(chunk) < 2:
                        continue
                    best = max(
                        (insts[i].sync_info.on_wait[0] for i in chunk),
                        key=lambda w: w.wait_value,
                    )
                    insts[chunk[0]].sync_info = mybir.SyncInfo(
                        on_wait=[best], on_update=[]
                    )
                    drop.update(chunk[1:])
            if drop:
                b.instructions[:] = [
                    inst for i, inst in enumerate(insts) if i not in drop
                ]


def _patch_minimal_teardown():
    """Tile's exit emits drain + 2 all-engine barriers + semaphore clears
    (~10us on the critical tail). The barriers/clears only matter for
    re-executing an already-loaded NEFF; each kernel() call loads fresh, so
    keep just the final drain (it carries the waits that guarantee the
    output DMAs completed)."""
    import concourse.tile as tile
    from concourse.vector_clock import ScopedClock

    if getattr(tile.TileContext._drain_and_barrier, "_minimal", False):
        return

    def _drain_and_barrier(self, tick_clock, wait_clock):
        drain_inst = self.nc.sync.drain()
        wait_clock.add_sem_waits(
            drain_inst.ins, ScopedClock({None: tick_clock.global_clock})
        )
        popped = self.nc._tile_sem_poison_stack.pop()
        assert popped is self._sem_poison

    _drain_and_barrier._minimal = True
    tile.TileContext._drain_and_barrier = _drain_and_barrier


def _build_nc():
    import concourse.bass as bass
    import concourse.mybir as mybir
    import concourse.tile as tile

    _patch_minimal_teardown()
    bf = mybir.dt.bfloat16
    f32 = mybir.dt.float32
    Exp = mybir.ActivationFunctionType.Exp

    nc = bass.Bass()
    qT2_e = nc.declare_dram_parameter("qT2", [HPC, 128, S], bf, isOutput=False)
    kT2_e = nc.declare_dram_parameter("kT2", [HPC, 128, KTP * 128], bf, isOutput=False)
    vh_e = nc.declare_dram_parameter("vh", [HPC, 128, KT * 65], bf, isOutput=False)
    mF_e = nc.declare_dram_parameter("maskF", [QC, 128, KT * 512], bf, isOutput=False)
    out_e = nc.declare_dram_parameter("out", [HPC, 65, S], bf, isOutput=True)

    with tile.TileContext(nc) as tc:
        with (
            tc.tile_pool(name="maskA", bufs=20) as maskA,
            tc.tile_pool(name="maskC", bufs=4) as maskC,
            tc.tile_pool(name="qp", bufs=4) as qp,
            tc.tile_pool(name="kp", bufs=4) as kp,
            tc.tile_pool(name="vp", bufs=4) as vp,
            tc.tile_pool(name="attA", bufs=6) as attAp,
            tc.tile_pool(name="attC", bufs=2) as attCp,
            tc.tile_pool(name="obp", bufs=2) as obp,
            tc.tile_pool(name="wp", bufs=1) as wp,
            tc.tile_pool(name="pA", bufs=2, space=bass.MemorySpace.PSUM) as pA,
            tc.tile_pool(name="pC", bufs=1, space=bass.MemorySpace.PSUM) as pC,
            tc.tile_pool(name="pO", bufs=1, space=bass.MemorySpace.PSUM) as pO,
        ):
            # touch the Exp table before any data arrives so the one-time
            # ACT_TABLE_LOAD overlaps the DMA ramp instead of the first tile
            warm_in = wp.tile([128, 8], bf, name="warm_in")
            nc.gpsimd.memset(warm_in[:], 0.0)
            warm_out = wp.tile([128, 8], bf, name="warm_out")
            nc.scalar.activation(warm_out[:], warm_in[:], Exp, scale=1.0)

            # DMA issue order = first-use order so the pipeline starts as
            # early as possible during the ramp: k0, first q half, first
            # mask tile, rest of head 0, then masks ahead of their heads.
            kts = [kp.tile([128, KTP * 128], bf, tag=f"k{h}", name=f"kt{h}", bufs=1)
                   for h in range(HPC)]
            qts = [qp.tile([128, S], bf, tag=f"q{h}", name=f"qt{h}", bufs=1)
                   for h in range(HPC)]
            vts = [vp.tile([128, KT * 65], bf, tag=f"v{h}", name=f"vt{h}", bufs=1)
                   for h in range(HPC)]
            mtiles = [[None] * 6 for _ in range(QC)]

            def load_mask(qq, t):
                if t < 5:
                    mt = maskA.tile([128, 1536], bf, tag=f"mA{qq}_{t}",
                                    name=f"mA{qq}_{t}", bufs=1)
                    nc.sync.dma_start(mt[:], mF_e[qq, :, t * 1536:(t + 1) * 1536])
                else:
                    mt = maskC.tile([128, 512], bf, tag=f"mC{qq}",
                                    name=f"mC{qq}", bufs=1)
                    nc.sync.dma_start(mt[:], mF_e[qq, :, 7680:8192])
                mtiles[qq][t] = mt

            nc.sync.dma_start(kts[0][:], kT2_e[0])
            nc.sync.dma_start(qts[0][:, 0:1024], qT2_e[0, :, 0:1024])
            load_mask(0, 0)
            nc.sync.dma_start(qts[0][:, 1024:2048], qT2_e[0, :, 1024:2048])
            nc.sync.dma_start(vts[0][:], vh_e[0])
            for t in range(1, 6):
                load_mask(0, t)
            for t in range(6):
                load_mask(1, t)
            nc.sync.dma_start(kts[1][:], kT2_e[1])
            nc.sync.dma_start(qts[1][:], qT2_e[1])
            nc.sync.dma_start(vts[1][:], vh_e[1])
            for t in range(6):
                load_mask(2, t)
            nc.sync.dma_start(kts[2][:], kT2_e[2])
            nc.sync.dma_start(qts[2][:], qT2_e[2])
            nc.sync.dma_start(vts[2][:], vh_e[2])
            for t in range(6):
                load_mask(3, t)
            nc.sync.dma_start(kts[3][:], kT2_e[3])
            nc.sync.dma_start(qts[3][:], qT2_e[3])
            nc.sync.dma_start(vts[3][:], vh_e[3])

            def s_matmul(s, kt_, qt, qq, j, col):
                rh = (j % 2) * 64
                nc.tensor.matmul(
                    s[:, col * 512:(col + 1) * 512],
                    kt_[rh:rh + 64, (j // 2) * 128:(j // 2 + 1) * 128],
                    qt[rh:rh + 64, qq * 512:(qq + 1) * 512],
                    start=True,
                    stop=True,
                )

            i16 = mybir.dt.int16
            Mult = mybir.AluOpType.mult
            Add = mybir.AluOpType.add

            for h in range(HPC):
                qt, kt_, vt = qts[h], kts[h], vts[h]
                for qq in range(QC):
                    g = h * QC + qq
                    o_ps = pO.tile([65, 512], f32, name=f"o_{h}_{qq}", tag="o")
                    sc = None
                    for t in range(5):
                        s = pA.tile([128, 1536], f32, name=f"s{h}_{qq}_{t}",
                                    tag="sA")
                        att = attAp.tile([128, 1536], bf, tag="attA",
                                         name="attA")
                        J = (3 * t, 3 * t + 1, 3 * t + 2)
                        for idx, j in enumerate(J):
                            s_matmul(s, kt_, qt, qq, j, idx)
                        if t == 1:
                            # k-tile 15's S matmul is emitted mid-group: its
                            # PSUM bank was freed by the previous group's
                            # last exp (2 ACT slots back by now, so no PE
                            # FIFO stall), and running it here keeps the
                            # group's tail chain off the critical path
                            # (accumulation order over k-tiles is arbitrary).
                            sc = pC.tile([128, 512], f32, name=f"sc{h}_{qq}",
                                         tag="sC")
                            s_matmul(sc, kt_, qt, qq, 15, 0)
                        if t == 2 and (g % 2 == 0 if OFFLOAD == 1 else OFFLOAD > 1):
                            nc.vector.tensor_scalar(
                                att[:].bitcast(i16), s[:],
                                SCALE * SCHR_A, SCHR_B, Mult, Add,
                            )
                        else:
                            nc.scalar.activation(att[:], s[:], Exp, scale=SCALE)
                        nc.vector.tensor_mul(att[:], att[:], mtiles[qq][t][:])
                        for idx, j in enumerate(J):
                            nc.tensor.matmul(
                                o_ps[:],
                                vt[:, j * 65:(j + 1) * 65],
                                att[:, idx * 512:(idx + 1) * 512],
                                start=(j == 0),
                                stop=False,
                            )
                    attc = attCp.tile([128, 512], bf, tag="attC", name="attC")
                    nc.scalar.activation(attc[:], sc[:], Exp, scale=SCALE)
                    nc.vector.tensor_mul(attc[:], attc[:], mtiles[qq][5][:])
                    nc.tensor.matmul(
                        o_ps[:],
                        vt[:, 15 * 65:16 * 65],
                        attc[:],
                        start=False,
                        stop=True,
                    )
                    o_sb = obp.tile([65, 512], bf, name=f"osb_{h}_{qq}", tag="osb")
                    nc.vector.tensor_copy(o_sb[:], o_ps[:])
                    nc.sync.dma_start(
                        out_e[h, :, qq * 512:(qq + 1) * 512], o_sb[:]
                    )
    import os

    _dedupe_and_split_waits(nc, max_waits=int(os.environ.get("KMAXW", "1")))
    rl = int(os.environ.get("KBUNDLE", "4"))
    if rl > 1:
        _bundle_scalar_dve_waits(nc, runlen=rl)
    return nc


def _core_inputs(q, k, v, mask, core):
    b = core // HPC
    h0 = (core % HPC) * HPC
    qh = q[b, h0:h0 + HPC].transpose(0, 2, 1)            # [4, 64, S]
    qT2 = np.concatenate([qh, qh], axis=1)               # [4, 128, S]
    kh = k[b, h0:h0 + HPC].transpose(0, 2, 1)            # [4, 64, S]
    kT2 = (
        kh.reshape(HPC, 64, KTP, 2, 128)
        .transpose(0, 3, 1, 2, 4)
        .reshape(HPC, 128, KTP * 128)
    )
    vv = v[b, h0:h0 + HPC]                               # [4, S, 64]
    vh = np.concatenate(
        [vv, np.ones((HPC, S, 1), dtype=np.float32)], axis=2
    )                                                    # [4, S, 65]
    vh = vh.reshape(HPC, KT, 128, 65).transpose(0, 2, 1, 3).reshape(HPC, 128, KT * 65)
    mT = np.ascontiguousarray(mask[b, 0].T)              # [k, q]
    mF = (
        mT.reshape(KT, 128, QC, 512)
        .transpose(2, 1, 0, 3)
        .reshape(QC, 128, KT * 512)
    )
    return {
        "qT2": np.ascontiguousarray(qT2).astype(BF16),
        "kT2": np.ascontiguousarray(kT2).astype(BF16),
        "vh": np.ascontiguousarray(vh).astype(BF16),
        "maskF": np.ascontiguousarray(mF).astype(BF16),
    }


def kernel(q, k, v, mask):
    global _NC
    from concourse.bass_utils import run_bass_kernel_spmd

    q = np.asarray(q, dtype=np.float32)
    k = np.asarray(k, dtype=np.float32)
    v = np.asarray(v, dtype=np.float32)
    mask = np.asarray(mask)

    in_maps = [_core_inputs(q, k, v, mask, c) for c in range(NCORES)]
    if _NC is None:
        _NC = _build_nc()

    res = run_bass_kernel_spmd(
        _NC, in_maps, core_ids=list(range(NCORES)), trace=TRACE
    )
    LAST["exec_time_ns"] = res.exec_time_ns
    LAST["results"] = res

    out = np.empty((B, H, S, Dh), dtype=np.float32)
    for c in range(NCORES):
        b = c // HPC
        h0 = (c % HPC) * HPC
        o = np.asarray(res.results[c]["out"], dtype=np.float32)  # [4, 65, S]
        sums = o[:, Dh:Dh + 1, :]                        # [4, 1, S]
        on = o[:, :Dh, :] / sums                         # [4, 64, S]
        out[b, h0:h0 + HPC] = on.transpose(0, 2, 1)
    return out


# revision 14
# speedup vs baseline: 1.2503x; 1.0575x over previous
"""Multi-head attention (B=2, H=16, S=2048, Dh=64) on 8 trn2 NeuronCores.

Sharding: core c handles batch c//4, heads (c%4)*4 .. +4. Each core computes
attention for its 4 (b,h) pairs independently (no collectives).

Device algorithm per (head, q-chunk of 512), k-tiles grouped in triples so
the ScalarE exp runs as 96 wide ACTIVATEs (FD 1536/512) instead of 128:
  S^T[k,q] = K @ Q^T   (bf16 matmuls; each k-tile's 64-deep contraction sits
                        in PE rows (j%2)*64 so adjacent matmuls row-tile)
  att      = exp(0.125 * S^T)       (ScalarE, PSUM->SBUF bf16 — the
                                     critical-path engine, ~1 elem/lane/cy)
  att     *= mask^T                 (VectorE bf16, in place, so the ACTIVATE
                                     needs only one deduped PE-sem wait)
  O^T[d,q] += [V | 1]^T @ att       (PSUM accumulation over all 16 k-tiles;
                                     the ones column makes row 64 the sums)
Host: normalize O^T rows by the sums row and transpose back to [S, Dh].
"""

import math
import os

import numpy as np
import ml_dtypes

TRACE = False
LAST = {}

# A-tiles whose exp runs on VectorE via the Schraudolph bit-trick
# (i16 = round(s*128*SCALE/ln2 + (127*128 - 7.4)), reinterpreted as bf16
# ~ exp(SCALE*s) with ~1.8% rms error) instead of on the saturated ScalarE.
# Value = offloaded A-tiles per 5-tile group (0..5).
OFFLOAD = int(os.environ.get("KOFFLOAD", "1"))
SCHR_A = 128.0 / math.log(2.0)
SCHR_B = 127.0 * 128.0 - 7.4

B, H, S, Dh = 2, 16, 2048, 64
HPC = 4            # heads per core
NCORES = 8
SCALE = 0.125      # Dh ** -0.5
KT = 16            # k tiles of 128
KTP = 8            # k tile pairs (kT2 column blocks)
QC = 4             # q chunks of 512
BF16 = ml_dtypes.bfloat16

_NC = None


def _dedupe_and_split_waits(nc, max_waits=1):
    """Collapse same-semaphore sem-ge-imm waits to the max threshold (they
    are monotonic counters, so the max implies the rest), then move any
    remaining excess onto NoOps inserted just before on the same engine —
    this container's walrus rejects >max_waits semaphore waits on one
    instruction (CoreV3 setupSyncWait "Too many sync wait commands")."""
    import concourse.mybir as mybir

    ctr = 0
    for f in nc.m.functions:
        for b in f.blocks:
            insts = b.instructions
            new = []
            for inst in insts:
                si = inst.sync_info
                waits = list(si.on_wait) if si else []
                if len(waits) > 1:
                    best, other = {}, []
                    for w in waits:
                        if w.wait_mode == "sem-ge-imm" and w.wait_reg is None:
                            k = (w.sync_type, w.id)
                            if k not in best or w.wait_value > best[k].wait_value:
                                best[k] = w
                        else:
                            other.append(w)
                    waits = list(best.values()) + other
                if len(waits) > max_waits:
                    for w in waits[:-max_waits]:
                        ctr += 1
                        new.append(
                            mybir.InstNoOp(
                                name=f"waitsplit-{ctr}",
                                engine=inst.engine,
                                ins=[],
                                outs=[],
                                sync_info=mybir.SyncInfo(on_wait=[w], on_update=[]),
                            )
                        )
                    waits = waits[-max_waits:]
                if si:
                    inst.sync_info = mybir.SyncInfo(
                        on_wait=waits, on_update=list(si.on_update)
                    )
                new.append(inst)
            insts[:] = new


def _bundle_scalar_dve_waits(nc, runlen=4):
    """The ScalarE stream is [NoOp(wait DVE>=mul(t-6)), ACTIVATE(wait PE)]
    per tile — the NoOp carries the att-buffer WAW wait split off by the
    1-wait-per-instruction limit. Since the DVE semaphore is monotonic and
    the att pool is 6 deep, one NoOp waiting for the max of the next
    `runlen` thresholds (producer: the mul 6-runlen tiles back, strictly
    earlier on every chain, so no cycle) covers all of them. Collapses 96
    sem-wait evaluations on the critical-path engine down to ~24."""
    import concourse.mybir as mybir

    # The DVE engine semaphore: the one TensorTensor instructions bump.
    from collections import Counter

    dve_sems = Counter()
    for f in nc.m.functions:
        for b in f.blocks:
            for inst in b.instructions:
                if isinstance(inst, mybir.InstTensorTensor) and inst.sync_info:
                    for u in inst.sync_info.on_update:
                        dve_sems[u.id] += 1
    if not dve_sems:
        return
    dve_id = dve_sems.most_common(1)[0][0]

    def is_dve_nop(inst):
        return (
            isinstance(inst, mybir.InstNoOp)
            and inst.sync_info
            and len(inst.sync_info.on_wait) == 1
            and not inst.sync_info.on_update
            and inst.sync_info.on_wait[0].id == dve_id
            and inst.sync_info.on_wait[0].wait_mode == "sem-ge-imm"
        )

    for f in nc.m.functions:
        for b in f.blocks:
            insts = b.instructions
            seq = [
                i for i, inst in enumerate(insts)
                if inst.engine == mybir.EngineType.Activation
            ]
            # maximal chains of consecutive [NoOp(dve-wait), ACTIVATE] pairs
            chains, cur, k = [], [], 0
            while k < len(seq):
                inst = insts[seq[k]]
                if (
                    is_dve_nop(inst)
                    and k + 1 < len(seq)
                    and isinstance(insts[seq[k + 1]], mybir.InstActivation)
                ):
                    cur.append(seq[k])
                    k += 2
                else:
                    if cur:
                        chains.append(cur)
                        cur = []
                    k += 1
            if cur:
                chains.append(cur)
            drop = set()
            for chain in chains:
                for c0 in range(0, len(chain), runlen):
                    chunk = chain[c0:c0 + runlen]
                    if len(chunk) < 2:
                        continue
                    best = max(
                        (insts[i].sync_info.on_wait[0] for i in chunk),
                        key=lambda w: w.wait_value,
                    )
                    insts[chunk[0]].sync_info = mybir.SyncInfo(
                        on_wait=[best], on_update=[]
                    )
                    drop.update(chunk[1:])
            if drop:
                b.instructions[:] = [
                    inst for i, inst in enumerate(insts) if i not in drop
                ]


def _patch_minimal_teardown():
    """Tile's exit emits drain + 2 all-engine barriers + semaphore clears
    (~10us on the critical tail). The barriers/clears only matter for
    re-executing an already-loaded NEFF; each kernel() call loads fresh, so
    keep just the final drain (it carries the waits that guarantee the
    output DMAs completed)."""
    import concourse.tile as tile
    from concourse.vector_clock import ScopedClock

    if getattr(tile.TileContext._drain_and_barrier, "_minimal", False):
        return

    def _drain_and_barrier(self, tick_clock, wait_clock):
        drain_inst = self.nc.sync.drain()
        wait_clock.add_sem_waits(
            drain_inst.ins, ScopedClock({None: tick_clock.global_clock})
        )
        popped = self.nc._tile_sem_poison_stack.pop()
        assert popped is self._sem_poison

    _drain_and_barrier._minimal = True
    tile.TileContext._drain_and_barrier = _drain_and_barrier


def _build_nc():
    import concourse.bass as bass
    import concourse.mybir as mybir
    import concourse.tile as tile

    _patch_minimal_teardown()
    bf = mybir.dt.bfloat16
    f32 = mybir.dt.float32
    Exp = mybir.ActivationFunctionType.Exp

    nc = bass.Bass()
    qT2_e = nc.declare_dram_parameter("qT2", [HPC, 128, S], bf, isOutput=False)
    kT2_e = nc.declare_dram_parameter("kT2", [HPC, 128, KTP * 128], bf, isOutput=False)
    vh_e = nc.declare_dram_parameter("vh", [HPC, 128, KT * 65], bf, isOutput=False)
    mF_e = nc.declare_dram_parameter("maskF", [QC, 128, KT * 512], bf, isOutput=False)
    out_e = nc.declare_dram_parameter("out", [HPC, 65, S], bf, isOutput=True)

    with tile.TileContext(nc) as tc:
        with (
            tc.tile_pool(name="maskA", bufs=20) as maskA,
            tc.tile_pool(name="maskC", bufs=4) as maskC,
            tc.tile_pool(name="qp", bufs=4) as qp,
            tc.tile_pool(name="kp", bufs=4) as kp,
            tc.tile_pool(name="vp", bufs=4) as vp,
            tc.tile_pool(name="attA", bufs=6) as attAp,
            tc.tile_pool(name="attC", bufs=2) as attCp,
            tc.tile_pool(name="obp", bufs=2) as obp,
            tc.tile_pool(name="wp", bufs=1) as wp,
            tc.tile_pool(name="pA", bufs=2, space=bass.MemorySpace.PSUM) as pA,
            tc.tile_pool(name="pC", bufs=1, space=bass.MemorySpace.PSUM) as pC,
            tc.tile_pool(name="pO", bufs=1, space=bass.MemorySpace.PSUM) as pO,
        ):
            # touch the Exp table before any data arrives so the one-time
            # ACT_TABLE_LOAD overlaps the DMA ramp instead of the first tile
            warm_in = wp.tile([128, 8], bf, name="warm_in")
            nc.gpsimd.memset(warm_in[:], 0.0)
            warm_out = wp.tile([128, 8], bf, name="warm_out")
            nc.scalar.activation(warm_out[:], warm_in[:], Exp, scale=1.0)

            # DMA issue order = first-use order so the pipeline starts as
            # early as possible during the ramp: k0, first q half, first
            # mask tile, rest of head 0, then masks ahead of their heads.
            kts = [kp.tile([128, KTP * 128], bf, tag=f"k{h}", name=f"kt{h}", bufs=1)
                   for h in range(HPC)]
            qts = [qp.tile([128, S], bf, tag=f"q{h}", name=f"qt{h}", bufs=1)
                   for h in range(HPC)]
            vts = [vp.tile([128, KT * 65], bf, tag=f"v{h}", name=f"vt{h}", bufs=1)
                   for h in range(HPC)]
            mtiles = [[None] * 6 for _ in range(QC)]

            def load_mask(qq, t):
                if t < 5:
                    mt = maskA.tile([128, 1536], bf, tag=f"mA{qq}_{t}",
                                    name=f"mA{qq}_{t}", bufs=1)
                    nc.sync.dma_start(mt[:], mF_e[qq, :, t * 1536:(t + 1) * 1536])
                else:
                    mt = maskC.tile([128, 512], bf, tag=f"mC{qq}",
                                    name=f"mC{qq}", bufs=1)
                    nc.sync.dma_start(mt[:], mF_e[qq, :, 7680:8192])
                mtiles[qq][t] = mt

            nc.sync.dma_start(kts[0][:], kT2_e[0])
            nc.sync.dma_start(qts[0][:, 0:1024], qT2_e[0, :, 0:1024])
            load_mask(0, 0)
            nc.sync.dma_start(qts[0][:, 1024:2048], qT2_e[0, :, 1024:2048])
            nc.sync.dma_start(vts[0][:], vh_e[0])
            for t in range(1, 6):
                load_mask(0, t)
            for t in range(6):
                load_mask(1, t)
            nc.sync.dma_start(kts[1][:], kT2_e[1])
            nc.sync.dma_start(qts[1][:], qT2_e[1])
            nc.sync.dma_start(vts[1][:], vh_e[1])
            for t in range(6):
                load_mask(2, t)
            nc.sync.dma_start(kts[2][:], kT2_e[2])
            nc.sync.dma_start(qts[2][:], qT2_e[2])
            nc.sync.dma_start(vts[2][:], vh_e[2])
            for t in range(6):
                load_mask(3, t)
            nc.sync.dma_start(kts[3][:], kT2_e[3])
            nc.sync.dma_start(qts[3][:], qT2_e[3])
            nc.sync.dma_start(vts[3][:], vh_e[3])

            def s_matmul(s, kt_, qt, qq, j, col):
                rh = (j % 2) * 64
                nc.tensor.matmul(
                    s[:, col * 512:(col + 1) * 512],
                    kt_[rh:rh + 64, (j // 2) * 128:(j // 2 + 1) * 128],
                    qt[rh:rh + 64, qq * 512:(qq + 1) * 512],
                    start=True,
                    stop=True,
                )

            i16 = mybir.dt.int16
            Mult = mybir.AluOpType.mult
            Add = mybir.AluOpType.add

            # Flat software pipeline over all 96 tiles (5 A-tiles of 3
            # k-tiles + 1 C-tile of k-tile 15 per (head, q-chunk) group).
            # Emission order per step n: ACT(n), mul(n), S(n+2), O(n) —
            # S matmuls always sit AHEAD of O matmuls in the PE FIFO, so
            # the exp stream never waits on the mul->O chain of the
            # previous tile, including across group boundaries.
            tiles = []
            for h in range(HPC):
                for qq in range(QC):
                    for t in range(5):
                        tiles.append(("A", h, qq, t))
                    tiles.append(("C", h, qq, 5))
            s_tiles = {}
            o_ps = {}

            def emit_S(n):
                kind, h, qq, t = tiles[n]
                qt, kt_ = qts[h], kts[h]
                if kind == "A":
                    s = pA.tile([128, 1536], f32, name=f"s{h}_{qq}_{t}",
                                tag="sA")
                    for idx, j in enumerate((3 * t, 3 * t + 1, 3 * t + 2)):
                        s_matmul(s, kt_, qt, qq, j, idx)
                else:
                    s = pC.tile([128, 512], f32, name=f"sc{h}_{qq}", tag="sC")
                    s_matmul(s, kt_, qt, qq, 15, 0)
                s_tiles[n] = s

            emit_S(0)
            emit_S(1)
            for n, (kind, h, qq, t) in enumerate(tiles):
                g = h * QC + qq
                s = s_tiles.pop(n)
                vt = vts[h]
                if kind == "A":
                    att = attAp.tile([128, 1536], bf, tag="attA", name="attA")
                    J = (3 * t, 3 * t + 1, 3 * t + 2)
                else:
                    att = attCp.tile([128, 512], bf, tag="attC", name="attC")
                    J = (15,)
                if (
                    kind == "A" and t == 2 and OFFLOAD
                    and (g % 2 == 0 or OFFLOAD > 1)
                ):
                    nc.vector.tensor_scalar(
                        att[:].bitcast(i16), s[:],
                        SCALE * SCHR_A, SCHR_B, Mult, Add,
                    )
                else:
                    nc.scalar.activation(att[:], s[:], Exp, scale=SCALE)
                nc.vector.tensor_mul(att[:], att[:], mtiles[qq][t][:])
                if n + 2 < len(tiles):
                    emit_S(n + 2)
                if t == 0:
                    o_ps[g] = pO.tile([65, 512], f32, name=f"o_{h}_{qq}",
                                      tag="o")
                for idx, j in enumerate(J):
                    nc.tensor.matmul(
                        o_ps[g][:],
                        vt[:, j * 65:(j + 1) * 65],
                        att[:, idx * 512:(idx + 1) * 512],
                        start=(j == 0),
                        stop=(j == 15),
                    )
                if kind == "C":
                    o_sb = obp.tile([65, 512], bf, name=f"osb_{h}_{qq}",
                                    tag="osb")
                    nc.vector.tensor_copy(o_sb[:], o_ps.pop(g)[:])
                    nc.sync.dma_start(
                        out_e[h, :, qq * 512:(qq + 1) * 512], o_sb[:]
                    )
    _dedupe_and_split_waits(nc, max_waits=int(os.environ.get("KMAXW", "1")))
    rl = int(os.environ.get("KBUNDLE", "4"))
    if rl > 1:
        _bundle_scalar_dve_waits(nc, runlen=rl)
    return nc


def _core_inputs(q, k, v, mask, core):
    b = core // HPC
    h0 = (core % HPC) * HPC
    qh = q[b, h0:h0 + HPC].transpose(0, 2, 1)            # [4, 64, S]
    qT2 = np.concatenate([qh, qh], axis=1)               # [4, 128, S]
    kh = k[b, h0:h0 + HPC].transpose(0, 2, 1)            # [4, 64, S]
    kT2 = (
        kh.reshape(HPC, 64, KTP, 2, 128)
        .transpose(0, 3, 1, 2, 4)
        .reshape(HPC, 128, KTP * 128)
    )
    vv = v[b, h0:h0 + HPC]                               # [4, S, 64]
    vh = np.concatenate(
        [vv, np.ones((HPC, S, 1), dtype=np.float32)], axis=2
    )                                                    # [4, S, 65]
    vh = vh.reshape(HPC, KT, 128, 65).transpose(0, 2, 1, 3).reshape(HPC, 128, KT * 65)
    mT = np.ascontiguousarray(mask[b, 0].T)              # [k, q]
    mF = (
        mT.reshape(KT, 128, QC, 512)
        .transpose(2, 1, 0, 3)
        .reshape(QC, 128, KT * 512)
    )
    return {
        "qT2": np.ascontiguousarray(qT2).astype(BF16),
        "kT2": np.ascontiguousarray(kT2).astype(BF16),
        "vh": np.ascontiguousarray(vh).astype(BF16),
        "maskF": np.ascontiguousarray(mF).astype(BF16),
    }


def kernel(q, k, v, mask):
    global _NC
    from concourse.bass_utils import run_bass_kernel_spmd

    q = np.asarray(q, dtype=np.float32)
    k = np.asarray(k, dtype=np.float32)
    v = np.asarray(v, dtype=np.float32)
    mask = np.asarray(mask)

    in_maps = [_core_inputs(q, k, v, mask, c) for c in range(NCORES)]
    if _NC is None:
        _NC = _build_nc()

    res = run_bass_kernel_spmd(
        _NC, in_maps, core_ids=list(range(NCORES)), trace=TRACE
    )
    LAST["exec_time_ns"] = res.exec_time_ns
    LAST["results"] = res

    out = np.empty((B, H, S, Dh), dtype=np.float32)
    for c in range(NCORES):
        b = c // HPC
        h0 = (c % HPC) * HPC
        o = np.asarray(res.results[c]["out"], dtype=np.float32)  # [4, 65, S]
        sums = o[:, Dh:Dh + 1, :]                        # [4, 1, S]
        on = o[:, :Dh, :] / sums                         # [4, 64, S]
        out[b, h0:h0 + HPC] = on.transpose(0, 2, 1)
    return out
